# revision 6
# baseline (speedup 1.0000x reference)
"""Causal self-attention on 8 TRN2 NeuronCores — v2 (pipelined, bf16).

Problem: x[2,2048,1024], wq/wk/wv/wo[1024,1024] (nn.Linear convention,
out = y @ W.T), H=16 heads, D=64, causal softmax, f32.

Sharding: tensor-parallel over heads x data-parallel over batch.
Core i handles batch b=i//4 and head group g=i%4 (4 heads each);
each core returns an f16 partial output projection and the host sums
the 4 partials per batch in f32.

v2 vs baseline: everything bf16 on device (half the HBM traffic,
1024-wide moving operands); the causal mask is folded into PSUM by a
small identity-matmul pre-write of -1e5 (start=True) that the scores
matmul accumulates onto (start=False), so exp feeds PV directly with
no DVE mask in the chain; attention runs one head at a time in two
query-span-pair passes (PSUM: 2x mg[128,1024] + 2x pv[65,512] +
2x proj[128,512] = exactly 8 banks) with independent projection /
output-projection matmul groups interleaved into the tensor queue as
fillers so the PE never idles long enough for the HAM clock gate to
drop it to 1.2 GHz (which is what made the baseline 2x slow).
"""

import sys

for _p in ("/opt/trn_rl_repo", "/root/.axon_site"):
    if _p not in sys.path:
        sys.path.insert(0, _p)

import numpy as np
import ml_dtypes

import concourse.bass as bass
import concourse.mybir as mybir
import concourse.tile as tile
from concourse import bacc
from concourse.bass_utils import run_bass_kernel_spmd

B, T, C, H = 2, 2048, 1024, 16
DH = C // H            # 64 head dim
HG = 4                 # heads per core
GW = HG * DH           # 256 features per head group
NB = T // 128          # 16 key chunks
NS = T // 512          # 4 spans
KC = C // 128          # 8 contraction chunks over C
SCALE = 1.0 / float(np.sqrt(DH))
MASKVAL = -1.0e5       # exp((s+MASKVAL)*SCALE) == 0 for any realistic s
N_CORES = 8

F32 = mybir.dt.float32
F16 = mybir.dt.float16
BF16 = mybir.dt.bfloat16
EXP = mybir.ActivationFunctionType.Exp
COPY = mybir.ActivationFunctionType.Copy


def build_nc():
    nc = bacc.Bacc("TRN2", target_bir_lowering=False, debug=False,
                   num_devices=N_CORES)
    xT = nc.declare_dram_parameter("xT", [C, T], BF16, isOutput=False)
    wqT = nc.declare_dram_parameter("wqT", [C, GW], BF16, isOutput=False)
    wkT = nc.declare_dram_parameter("wkT", [C, GW], BF16, isOutput=False)
    wvT = nc.declare_dram_parameter("wvT", [C, GW], BF16, isOutput=False)
    woT = nc.declare_dram_parameter("woT", [GW, C], BF16, isOutput=False)
    outT = nc.declare_dram_parameter("outT", [C, T], F16, isOutput=True)
    s_dram = nc.dram_tensor("s_scratch", [HG, NS, 512], F32)

    with tile.TileContext(nc) as tc:
        with tc.tile_pool(name="pers", bufs=1) as pers, \
             tc.tile_pool(name="PJ", bufs=2, space="PSUM") as PJ, \
             tc.tile_pool(name="MG", bufs=2, space="PSUM") as MG, \
             tc.tile_pool(name="PV", bufs=1, space="PSUM") as PVP, \
             tc.tile_pool(name="PT", bufs=3) as PT, \
             tc.tile_pool(name="NR", bufs=3) as NR, \
             tc.tile_pool(name="OT", bufs=2) as OT:
            # ---- persistent SBUF; DMAs in consumption order ----
            wk_t = [pers.tile([128, GW], BF16, tag=f"wk{i}", name=f"wk{i}")
                    for i in range(KC)]
            wq_t = [pers.tile([128, GW], BF16, tag=f"wq{i}", name=f"wq{i}")
                    for i in range(KC)]
            for i in range(KC):
                nc.gpsimd.dma_start(out=wk_t[i], in_=wkT[i * 128:(i + 1) * 128, :])
            for i in range(KC):
                nc.gpsimd.dma_start(out=wq_t[i], in_=wqT[i * 128:(i + 1) * 128, :])
            xts = [pers.tile([128, T], BF16, tag=f"xT{i}", name=f"xT{i}")
                   for i in range(KC)]
            for hhalf in range(2):
                cols = slice(hhalf * 1024, (hhalf + 1) * 1024)
                for i in range(KC):
                    nc.sync.dma_start(out=xts[i][:, cols],
                                      in_=xT[i * 128:(i + 1) * 128, cols])
            wv_t = [pers.tile([128, GW], BF16, tag=f"wv{i}", name=f"wv{i}")
                    for i in range(KC)]
            for i in range(KC):
                nc.gpsimd.dma_start(out=wv_t[i], in_=wvT[i * 128:(i + 1) * 128, :])
            wo_t = [pers.tile([128, C], BF16, tag=f"wo{j}", name=f"wo{j}")
                    for j in range(2)]
            for j in range(2):
                nc.gpsimd.dma_start(out=wo_t[j], in_=woT[j * 128:(j + 1) * 128, :])

            qts = [pers.tile([128, T], BF16, tag=f"qT{m}", name=f"qT{m}")
                   for m in range(2)]
            kts = [pers.tile([128, T], BF16, tag=f"kT{m}", name=f"kT{m}")
                   for m in range(2)]
            yts = [pers.tile([128, T], BF16, tag=f"yT{m}", name=f"yT{m}")
                   for m in range(2)]
            vts = [pers.tile([128, HG * 65], BF16, tag=f"V{tb}", name=f"V{tb}")
                   for tb in range(NB)]

            # identity (bf16) and causal-mask pre-write tile:
            # maskM[i,j] = MASKVAL where j<i (query j < key i) else 0
            ident = pers.tile([128, 128], BF16, tag="ident", name="ident")
            nc.gpsimd.memset(ident, 1.0)
            nc.gpsimd.affine_select(
                out=ident, in_=ident, compare_op=mybir.AluOpType.is_ge,
                fill=0.0, base=0, pattern=[[1, 128]], channel_multiplier=-1)
            nc.gpsimd.affine_select(
                out=ident, in_=ident, compare_op=mybir.AluOpType.is_ge,
                fill=0.0, base=0, pattern=[[-1, 128]], channel_multiplier=1)
            maskM = pers.tile([128, 128], BF16, tag="maskM", name="maskM")
            nc.gpsimd.memset(maskM, MASKVAL)
            nc.gpsimd.affine_select(
                out=maskM, in_=maskM, compare_op=mybir.AluOpType.is_ge,
                fill=0.0, base=-1, pattern=[[-1, 128]], channel_multiplier=1)
            ones4 = pers.tile([128, 4], BF16, tag="ones4", name="ones4")
            for j in range(4):
                nc.scalar.activation(
                    out=ones4[:, j:j + 1],
                    in_=nc.const_aps.tensor(1.0, [128, 1]), func=COPY)

            # ---- emission helpers ----
            def qk_group(wt, dst, m, s):
                """One projection accumulation group: dst[:, s*512:...]"""
                ps = PJ.tile([128, 512], F32, tag="pj", name="pj")
                for k in range(KC):
                    nc.tensor.matmul(
                        ps, wt[k][:, m * 128:(m + 1) * 128],
                        xts[k][:, s * 512:(s + 1) * 512],
                        start=(k == 0), stop=(k == KC - 1))
                nc.vector.tensor_copy(
                    out=dst[:, s * 512:(s + 1) * 512], in_=ps)

            def v_group(tb):
                """V for key chunk tb in natural [t, d] layout + ones col."""
                vps = PJ.tile([128, 512], F32, tag="pj", name="pj")
                for k in range(KC):
                    nc.tensor.matmul(
                        vps[:, 0:GW], xts[k][:, tb * 128:(tb + 1) * 128],
                        wv_t[k], start=(k == 0), stop=(k == KC - 1))
                vt = vts[tb]
                for h in range(HG):
                    nc.vector.tensor_copy(
                        out=vt[:, h * 65:h * 65 + 64],
                        in_=vps[:, h * 64:(h + 1) * 64])
                nc.vector.tensor_copy(
                    out=vt.rearrange("p (h c) -> p h c", c=65)[:, :, 64],
                    in_=ones4)

            def op_group(m, gs):
                """Output projection for block m, span gs -> OT staging."""
                op = PJ.tile([128, 512], F32, tag="pj", name="pj")
                for j in range(2):
                    nc.tensor.matmul(
                        op, wo_t[j][:, m * 128:(m + 1) * 128],
                        yts[j][:, gs * 512:(gs + 1) * 512],
                        start=(j == 0), stop=(j == 1))
                half = gs // 2
                ot = ot_tiles[m][half]
                if ot is None:
                    ot = OT.tile([128, 1024], F16, tag="ot", name="ot")
                    ot_tiles[m][half] = ot
                nc.vector.tensor_copy(
                    out=ot[:, (gs % 2) * 512:(gs % 2 + 1) * 512], in_=op)
                if gs % 2 == 1:
                    nc.sync.dma_start(
                        out=outT[m * 128:(m + 1) * 128,
                                 half * 1024:(half + 1) * 1024],
                        in_=ot)
                    ot_tiles[m][half] = None

            ot_tiles = [[None, None] for _ in range(8)]

            def norm_span(h, gs, pvt):
                """Normalize completed span: yts <- pv[0:64] / rowsum."""
                m, po = h // 2, (h % 2) * 64
                yv = NR.tile([65, 512], F32, tag="yv", name="yv")
                nc.vector.tensor_copy(out=yv, in_=pvt)
                nc.gpsimd.dma_start(out=s_dram[h, gs, :], in_=yv[64:65, :])
                sb = NR.tile([64, 512], F32, tag="sb", name="sb")
                ssl = s_dram[h, gs, :]
                nc.gpsimd.dma_start(
                    out=sb,
                    in_=bass.AP(tensor=ssl.tensor, offset=ssl.offset,
                                ap=[[0, 64]] + list(ssl.ap)))
                rb = NR.tile([64, 512], F32, tag="rb", name="rb")
                nc.vector.reciprocal(out=rb, in_=sb)
                nc.vector.tensor_mul(
                    out=yts[m][po:po + 64, gs * 512:(gs + 1) * 512],
                    in0=yv[0:64, :], in1=rb)

            # ---- attention for one head, one query-span-pair pass ----
            def attn_pass(h, qpass, fillers):
                m, po = h // 2, (h % 2) * 64
                qt, kt = qts[m], kts[m]
                qbase = qpass * 1024
                ki_hi = 8 if qpass == 0 else 16
                pva = PVP.tile([65, 512], F32, tag="pva", name="pva")
                pvb = PVP.tile([65, 512], F32, tag="pvb", name="pvb")
                pv = (pva, pvb)
                for ki in range(ki_hi):
                    kcol = 128 * ki
                    w0 = max(0, kcol - qbase)
                    diag = kcol >= qbase
                    mg = MG.tile([128, 1024], F32, tag="mg", name="mg")
                    if diag:
                        nc.tensor.matmul(mg[:, w0:w0 + 128], ident, maskM,
                                         start=True, stop=False)
                        nc.tensor.matmul(
                            mg[:, w0:w0 + 128],
                            kt[po:po + 64, kcol:kcol + 128],
                            qt[po:po + 64, qbase + w0:qbase + w0 + 128],
                            start=False, stop=True)
                        segs = []
                        a = w0 + 128
                        if a < 512:
                            segs.append((a, 512))
                        if max(a, 512) < 1024:
                            segs.append((max(a, 512), 1024))
                    else:
                        segs = [(0, 512), (512, 1024)]
                    for (lo, hi) in segs:
                        nc.tensor.matmul(
                            mg[:, lo:hi],
                            kt[po:po + 64, kcol:kcol + 128],
                            qt[po:po + 64, qbase + lo:qbase + hi],
                            start=True, stop=True)
                    pt = PT.tile([128, 1024], BF16, tag="pt", name="pt")
                    nc.scalar.activation(out=pt[:, w0:1024], in_=mg[:, w0:1024],
                                         func=EXP, scale=SCALE)
                    # filler goes between scores and PV to cover exp latency
                    if fillers:
                        fillers.pop(0)()
                    for sp in range(2):
                        gs = qpass * 2 + sp
                        last_ki = 4 * gs + 3
                        if ki > last_ki:
                            continue
                        lo = sp * 512
                        l = max(lo, w0)
                        if l >= lo + 512:
                            continue
                        nc.tensor.matmul(
                            pv[sp][:, l - lo:512],
                            vts[ki][:, h * 65:(h + 1) * 65],
                            pt[:, l:lo + 512],
                            start=(ki == 0), stop=(ki == last_ki))
                        if ki == last_ki:
                            norm_span(h, gs, pv[sp])

            # ---- schedule ----
            # QK projections for head pair 0
            for s in range(NS):
                qk_group(wk_t, kts[0], 0, s)
            for s in range(NS):
                qk_group(wq_t, qts[0], 0, s)
            v_group(0)
            v_group(1)

            fill_h0a = [(lambda tb: (lambda: v_group(tb)))(tb)
                        for tb in range(2, 10)]
            fill_h0b = [(lambda tb: (lambda: v_group(tb)))(tb)
                        for tb in range(10, 16)]
            fill_h0b += [(lambda s: (lambda: qk_group(wk_t, kts[1], 1, s)))(s)
                         for s in range(NS)]
            fill_h0b += [(lambda s: (lambda: qk_group(wq_t, qts[1], 1, s)))(s)
                         for s in range(2)]
            fill_h1a = [(lambda s: (lambda: qk_group(wq_t, qts[1], 1, s)))(s)
                        for s in range(2, NS)]
            fill_h3b = [(lambda m, gs: (lambda: op_group(m, gs)))(m, gs)
                        for m in range(8) for gs in range(2)]

            attn_pass(0, 0, fill_h0a)
            attn_pass(0, 1, fill_h0b)
            attn_pass(1, 0, fill_h1a)
            attn_pass(1, 1, [])
            attn_pass(2, 0, [])
            attn_pass(2, 1, [])
            attn_pass(3, 0, [])
            attn_pass(3, 1, fill_h3b)

            # tail: output projection spans 2,3
            for m in range(8):
                for gs in range(2, NS):
                    op_group(m, gs)
    nc.compile()
    return nc


_NC_CACHE = None


def _get_nc():
    global _NC_CACHE
    if _NC_CACHE is None:
        _NC_CACHE = build_nc()
    return _NC_CACHE


def make_in_maps(x, wq, wk, wv, wo):
    BF = ml_dtypes.bfloat16
    x = np.asarray(x, dtype=np.float32)
    wq = np.asarray(wq, dtype=np.float32)
    wk = np.asarray(wk, dtype=np.float32)
    wv = np.asarray(wv, dtype=np.float32)
    wo = np.asarray(wo, dtype=np.float32)
    in_maps = []
    for core in range(N_CORES):
        b, g = core // HG, core % HG
        rows = slice(g * GW, (g + 1) * GW)
        in_maps.append({
            "xT": np.ascontiguousarray(x[b].T).astype(BF),
            "wqT": np.ascontiguousarray(wq[rows, :].T).astype(BF),
            "wkT": np.ascontiguousarray(wk[rows, :].T).astype(BF),
            "wvT": np.ascontiguousarray(wv[rows, :].T).astype(BF),
            "woT": np.ascontiguousarray(wo[:, rows].T).astype(BF),
        })
    return in_maps


def run(x, wq, wk, wv, wo, trace=False, tmpdir=None):
    nc = _get_nc()
    in_maps = make_in_maps(x, wq, wk, wv, wo)
    res = run_bass_kernel_spmd(nc, in_maps, core_ids=list(range(N_CORES)),
                               trace=trace, tmpdir=tmpdir)
    out = np.zeros((B, T, C), dtype=np.float32)
    for core in range(N_CORES):
        out[core // HG] += res.results[core]["outT"].T.astype(np.float32)
    return out, res


def kernel(x, wq, wk, wv, wo):
    out, _ = run(x, wq, wk, wv, wo)
    return out


# revision 8
# speedup vs baseline: 1.1202x; 1.1202x over previous
"""Causal self-attention on 8 TRN2 NeuronCores — v3 (pipelined, bf16).

Problem: x[2,2048,1024], wq/wk/wv/wo[1024,1024] (nn.Linear convention,
out = y @ W.T), H=16 heads, D=64, causal softmax, f32.

Sharding: tensor-parallel over heads x data-parallel over batch.
Core i handles batch b=i//4 and head group g=i%4 (4 heads each);
each core returns an f16 partial output projection and the host sums
the 4 partials per batch in f32.

Design: everything bf16 on device; causal mask folded into PSUM by an
identity-matmul pre-write of -1e5 (start=True) that the scores matmul
accumulates onto (start=False), so exp feeds PV directly; attention
runs one head at a time in two query-span-pair passes, ordered
h0A..h3A then h0B..h3B so independent matmul work exists everywhere:
V/QK projections fill the pass-A region, output-projection spans 0/1
fill the pass-B region (their Y rows complete after the A region).
PV runs one ki-step behind scores so ScalarE exp is never gated by
the tensor queue. Softmax 1/sum uses reciprocal_approx_fast on the
DMA-broadcast row (the exact DVE reciprocal on [64,512] costs 3.3us
per call and froze the pipeline in v2). PSUM: 2x mg[128,1024] +
2x pv[65,512] + 2x proj[128,512] = exactly 8 banks.
"""

import sys

for _p in ("/opt/trn_rl_repo", "/root/.axon_site"):
    if _p not in sys.path:
        sys.path.insert(0, _p)

import numpy as np
import ml_dtypes

import concourse.bass as bass
import concourse.mybir as mybir
import concourse.tile as tile
from concourse import bacc
from concourse.bass_utils import run_bass_kernel_spmd

B, T, C, H = 2, 2048, 1024, 16
DH = C // H            # 64 head dim
HG = 4                 # heads per core
GW = HG * DH           # 256 features per head group
NB = T // 128          # 16 key chunks
NS = T // 512          # 4 spans
KC = C // 128          # 8 contraction chunks over C
SCALE = 1.0 / float(np.sqrt(DH))
MASKVAL = -1.0e5       # exp((s+MASKVAL)*SCALE) == 0 for any realistic s
N_CORES = 8

F32 = mybir.dt.float32
F16 = mybir.dt.float16
BF16 = mybir.dt.bfloat16
EXP = mybir.ActivationFunctionType.Exp
COPY = mybir.ActivationFunctionType.Copy


def build_nc():
    nc = bacc.Bacc("TRN2", target_bir_lowering=False, debug=False,
                   num_devices=N_CORES)
    xT = nc.declare_dram_parameter("xT", [C, T], BF16, isOutput=False)
    wqT = nc.declare_dram_parameter("wqT", [C, GW], BF16, isOutput=False)
    wkT = nc.declare_dram_parameter("wkT", [C, GW], BF16, isOutput=False)
    wvT = nc.declare_dram_parameter("wvT", [C, GW], BF16, isOutput=False)
    woT = nc.declare_dram_parameter("woT", [GW, C], BF16, isOutput=False)
    outT = nc.declare_dram_parameter("outT", [C, T], F16, isOutput=True)
    s_dram = nc.dram_tensor("s_scratch", [HG, NS, 512], F32)

    with tile.TileContext(nc) as tc:
        with tc.tile_pool(name="pers", bufs=1) as pers, \
             tc.tile_pool(name="PJ", bufs=2, space="PSUM") as PJ, \
             tc.tile_pool(name="MG", bufs=2, space="PSUM") as MG, \
             tc.tile_pool(name="PV", bufs=1, space="PSUM") as PVP, \
             tc.tile_pool(name="PT", bufs=3) as PT, \
             tc.tile_pool(name="NR", bufs=3) as NR, \
             tc.tile_pool(name="OT", bufs=2) as OT:
            # ---- persistent SBUF; DMAs in consumption order ----
            wk_t = [pers.tile([128, GW], BF16, tag=f"wk{i}", name=f"wk{i}")
                    for i in range(KC)]
            wq_t = [pers.tile([128, GW], BF16, tag=f"wq{i}", name=f"wq{i}")
                    for i in range(KC)]
            for i in range(KC):
                nc.gpsimd.dma_start(out=wk_t[i], in_=wkT[i * 128:(i + 1) * 128, :])
            for i in range(KC):
                nc.gpsimd.dma_start(out=wq_t[i], in_=wqT[i * 128:(i + 1) * 128, :])
            xts = [pers.tile([128, T], BF16, tag=f"xT{i}", name=f"xT{i}")
                   for i in range(KC)]
            for hhalf in range(2):
                cols = slice(hhalf * 1024, (hhalf + 1) * 1024)
                for i in range(KC):
                    nc.sync.dma_start(out=xts[i][:, cols],
                                      in_=xT[i * 128:(i + 1) * 128, cols])
            wv_t = [pers.tile([128, GW], BF16, tag=f"wv{i}", name=f"wv{i}")
                    for i in range(KC)]
            for i in range(KC):
                nc.gpsimd.dma_start(out=wv_t[i], in_=wvT[i * 128:(i + 1) * 128, :])
            wo_t = [pers.tile([128, C], BF16, tag=f"wo{j}", name=f"wo{j}")
                    for j in range(2)]
            for j in range(2):
                nc.gpsimd.dma_start(out=wo_t[j], in_=woT[j * 128:(j + 1) * 128, :])

            qts = [pers.tile([128, T], BF16, tag=f"qT{m}", name=f"qT{m}")
                   for m in range(2)]
            kts = [pers.tile([128, T], BF16, tag=f"kT{m}", name=f"kT{m}")
                   for m in range(2)]
            yts = [pers.tile([128, T], BF16, tag=f"yT{m}", name=f"yT{m}")
                   for m in range(2)]
            vts = [pers.tile([128, HG * 65], BF16, tag=f"V{tb}", name=f"V{tb}")
                   for tb in range(NB)]

            # identity (bf16) and causal-mask pre-write tile:
            # maskM[i,j] = MASKVAL where j<i (query j < key i) else 0
            ident = pers.tile([128, 128], BF16, tag="ident", name="ident")
            nc.gpsimd.memset(ident, 1.0)
            nc.gpsimd.affine_select(
                out=ident, in_=ident, compare_op=mybir.AluOpType.is_ge,
                fill=0.0, base=0, pattern=[[1, 128]], channel_multiplier=-1)
            nc.gpsimd.affine_select(
                out=ident, in_=ident, compare_op=mybir.AluOpType.is_ge,
                fill=0.0, base=0, pattern=[[-1, 128]], channel_multiplier=1)
            maskM = pers.tile([128, 128], BF16, tag="maskM", name="maskM")
            nc.gpsimd.memset(maskM, MASKVAL)
            nc.gpsimd.affine_select(
                out=maskM, in_=maskM, compare_op=mybir.AluOpType.is_ge,
                fill=0.0, base=-1, pattern=[[-1, 128]], channel_multiplier=1)
            ones4 = pers.tile([128, 4], BF16, tag="ones4", name="ones4")
            for j in range(4):
                nc.scalar.activation(
                    out=ones4[:, j:j + 1],
                    in_=nc.const_aps.tensor(1.0, [128, 1]), func=COPY)
            # ones columns of the V tiles are static: write them once
            for tb in range(NB):
                nc.vector.tensor_copy(
                    out=vts[tb].rearrange("p (h c) -> p h c", c=65)[:, :, 64],
                    in_=ones4)

            # ---- emission helpers ----
            def qk_group(wt, dst, m, s):
                """One projection accumulation group: dst[:, s*512:...]"""
                ps = PJ.tile([128, 512], F32, tag="pj", name="pj")
                for k in range(KC):
                    nc.tensor.matmul(
                        ps, wt[k][:, m * 128:(m + 1) * 128],
                        xts[k][:, s * 512:(s + 1) * 512],
                        start=(k == 0), stop=(k == KC - 1))
                nc.vector.tensor_copy(
                    out=dst[:, s * 512:(s + 1) * 512], in_=ps)

            def v_group(tb):
                """V for key chunk tb in natural [t, d] layout (strided cast)."""
                vps = PJ.tile([128, 512], F32, tag="pj", name="pj")
                for k in range(KC):
                    nc.tensor.matmul(
                        vps[:, 0:GW], xts[k][:, tb * 128:(tb + 1) * 128],
                        wv_t[k], start=(k == 0), stop=(k == KC - 1))
                nc.vector.tensor_copy(
                    out=vts[tb].rearrange("p (h c) -> p h c", c=65)[:, :, 0:64],
                    in_=vps.rearrange("p (h c) -> p h c", c=64)[:, 0:4, :])

            def op_group(m, gs):
                """Output projection for block m, span gs -> OT staging."""
                op = PJ.tile([128, 512], F32, tag="pj", name="pj")
                for j in range(2):
                    nc.tensor.matmul(
                        op, wo_t[j][:, m * 128:(m + 1) * 128],
                        yts[j][:, gs * 512:(gs + 1) * 512],
                        start=(j == 0), stop=(j == 1))
                half = gs // 2
                ot = ot_tiles[m][half]
                if ot is None:
                    ot = OT.tile([128, 1024], F16, tag="ot", name="ot")
                    ot_tiles[m][half] = ot
                nc.vector.tensor_copy(
                    out=ot[:, (gs % 2) * 512:(gs % 2 + 1) * 512], in_=op)
                if gs % 2 == 1:
                    nc.sync.dma_start(
                        out=outT[m * 128:(m + 1) * 128,
                                 half * 1024:(half + 1) * 1024],
                        in_=ot)
                    ot_tiles[m][half] = None

            ot_tiles = [[None, None] for _ in range(8)]

            def norm_span(h, gs, pvt):
                """Normalize completed span: yts <- pv[0:64] / rowsum."""
                m, po = h // 2, (h % 2) * 64
                yv = NR.tile([65, 512], F32, tag="yv", name="yv")
                nc.vector.tensor_copy(out=yv, in_=pvt)
                nc.gpsimd.dma_start(out=s_dram[h, gs, :], in_=yv[64:65, :])
                sb = NR.tile([64, 512], F32, tag="sb", name="sb")
                ssl = s_dram[h, gs, :]
                nc.gpsimd.dma_start(
                    out=sb,
                    in_=bass.AP(tensor=ssl.tensor, offset=ssl.offset,
                                ap=[[0, 64]] + list(ssl.ap)))
                rb = NR.tile([64, 512], F32, tag="rb", name="rb")
                nc.vector.reciprocal_approx_fast(out=rb, in_=sb)
                nc.vector.tensor_mul(
                    out=yts[m][po:po + 64, gs * 512:(gs + 1) * 512],
                    in0=yv[0:64, :], in1=rb)

            # ---- attention for one head, one query-span-pair pass.
            # PV trails scores by one ki step so exp never gates the
            # tensor queue (filler + PV(n-1) + S(n+1) run under exp(n)).
            def attn_pass(h, qpass, fillers):
                m, po = h // 2, (h % 2) * 64
                qt, kt = qts[m], kts[m]
                qbase = qpass * 1024
                ki_hi = 8 if qpass == 0 else 16
                pva = PVP.tile([65, 512], F32, tag="pva", name="pva")
                pvb = PVP.tile([65, 512], F32, tag="pvb", name="pvb")
                pv = (pva, pvb)
                pend = None  # (ki, pt) awaiting PV emission

                def emit_pv(ki, pt):
                    for sp in range(2):
                        gs = qpass * 2 + sp
                        last_ki = 4 * gs + 3
                        if ki > last_ki:
                            continue
                        lo = sp * 512
                        l = max(lo, max(0, 128 * ki - qbase))
                        if l >= lo + 512:
                            continue
                        nc.tensor.matmul(
                            pv[sp][:, l - lo:512],
                            vts[ki][:, h * 65:(h + 1) * 65],
                            pt[:, l:lo + 512],
                            start=(ki == 0), stop=(ki == last_ki))
                        if ki == last_ki:
                            norm_span(h, gs, pv[sp])

                for ki in range(ki_hi):
                    kcol = 128 * ki
                    w0 = max(0, kcol - qbase)
                    diag = kcol >= qbase
                    mg = MG.tile([128, 1024], F32, tag="mg", name="mg")
                    if diag:
                        nc.tensor.matmul(mg[:, w0:w0 + 128], ident, maskM,
                                         start=True, stop=False)
                        nc.tensor.matmul(
                            mg[:, w0:w0 + 128],
                            kt[po:po + 64, kcol:kcol + 128],
                            qt[po:po + 64, qbase + w0:qbase + w0 + 128],
                            start=False, stop=True)
                        segs = []
                        a = w0 + 128
                        if a < 512:
                            segs.append((a, 512))
                        if max(a, 512) < 1024:
                            segs.append((max(a, 512), 1024))
                    else:
                        segs = [(0, 512), (512, 1024)]
                    for (lo, hi) in segs:
                        nc.tensor.matmul(
                            mg[:, lo:hi],
                            kt[po:po + 64, kcol:kcol + 128],
                            qt[po:po + 64, qbase + lo:qbase + hi],
                            start=True, stop=True)
                    pt = PT.tile([128, 1024], BF16, tag="pt", name="pt")
                    nc.scalar.activation(out=pt[:, w0:1024], in_=mg[:, w0:1024],
                                         func=EXP, scale=SCALE)
                    if fillers:
                        fillers.pop(0)()
                    if pend is not None:
                        emit_pv(*pend)
                    pend = (ki, pt)
                if pend is not None:
                    emit_pv(*pend)

            # ---- schedule ----
            # QK projections for head pair 0
            for s in range(NS):
                qk_group(wk_t, kts[0], 0, s)
            for s in range(NS):
                qk_group(wq_t, qts[0], 0, s)
            v_group(0)
            v_group(1)

            # pass-A region fillers. Dependencies: h0-A consumes V(ki) at
            # step ki+1 so V(2..7) must ride h0-A itself; QK for head
            # pair 1 must complete before h2-A's first scores matmul;
            # V(8..15) is only consumed in the pass-B region.
            fa_lists = [
                [(lambda tb: (lambda: v_group(tb)))(tb) for tb in range(2, 8)],
                [(lambda s: (lambda: qk_group(wk_t, kts[1], 1, s)))(s)
                 for s in range(NS)] +
                [(lambda s: (lambda: qk_group(wq_t, qts[1], 1, s)))(s)
                 for s in range(NS)],
                [(lambda tb: (lambda: v_group(tb)))(tb) for tb in range(8, 14)],
                [(lambda tb: (lambda: v_group(tb)))(tb) for tb in range(14, 16)],
            ]

            for h in range(HG):
                attn_pass(h, 0, fa_lists[h])

            # pass-B region fillers: out-proj spans 0,1 (one per 4 steps)
            fb = [(lambda m, gs: (lambda: op_group(m, gs)))(m, gs)
                  for m in range(8) for gs in range(2)]
            fb_lists = []
            fi = 0
            for h in range(HG):
                sub = []
                for step in range(16):
                    if step % 4 == 1 and fi < len(fb):
                        sub.append(fb[fi])
                        fi += 1
                fb_lists.append(sub)

            # interleave: a pass consumes its filler list one per ki step
            for h in range(HG):
                attn_pass(h, 1, fb_lists[h])

            # tail: output projection spans 2,3
            for m in range(8):
                for gs in range(2, NS):
                    op_group(m, gs)
    nc.compile()
    return nc


_NC_CACHE = None


def _get_nc():
    global _NC_CACHE
    if _NC_CACHE is None:
        _NC_CACHE = build_nc()
    return _NC_CACHE


def make_in_maps(x, wq, wk, wv, wo):
    BF = ml_dtypes.bfloat16
    x = np.asarray(x, dtype=np.float32)
    wq = np.asarray(wq, dtype=np.float32)
    wk = np.asarray(wk, dtype=np.float32)
    wv = np.asarray(wv, dtype=np.float32)
    wo = np.asarray(wo, dtype=np.float32)
    in_maps = []
    for core in range(N_CORES):
        b, g = core // HG, core % HG
        rows = slice(g * GW, (g + 1) * GW)
        in_maps.append({
            "xT": np.ascontiguousarray(x[b].T).astype(BF),
            "wqT": np.ascontiguousarray(wq[rows, :].T).astype(BF),
            "wkT": np.ascontiguousarray(wk[rows, :].T).astype(BF),
            "wvT": np.ascontiguousarray(wv[rows, :].T).astype(BF),
            "woT": np.ascontiguousarray(wo[:, rows].T).astype(BF),
        })
    return in_maps


def run(x, wq, wk, wv, wo, trace=False, tmpdir=None):
    nc = _get_nc()
    in_maps = make_in_maps(x, wq, wk, wv, wo)
    res = run_bass_kernel_spmd(nc, in_maps, core_ids=list(range(N_CORES)),
                               trace=trace, tmpdir=tmpdir)
    out = np.zeros((B, T, C), dtype=np.float32)
    for core in range(N_CORES):
        out[core // HG] += res.results[core]["outT"].T.astype(np.float32)
    return out, res


def kernel(x, wq, wk, wv, wo):
    out, _ = run(x, wq, wk, wv, wo)
    return out


# revision 16
# speedup vs baseline: 1.1314x; 1.0100x over previous
"""Causal self-attention on 8 TRN2 NeuronCores — v3 (pipelined, bf16).

Problem: x[2,2048,1024], wq/wk/wv/wo[1024,1024] (nn.Linear convention,
out = y @ W.T), H=16 heads, D=64, causal softmax, f32.

Sharding: tensor-parallel over heads x data-parallel over batch.
Core i handles batch b=i//4 and head group g=i%4 (4 heads each);
each core returns an f16 partial output projection and the host sums
the 4 partials per batch in f32.

Design: everything bf16 on device; causal mask folded into PSUM by an
identity-matmul pre-write of -1e5 (start=True) that the scores matmul
accumulates onto (start=False), so exp feeds PV directly; attention
runs one head at a time in two query-span-pair passes, ordered
h0A..h3A then h0B..h3B so independent matmul work exists everywhere:
V/QK projections fill the pass-A region, output-projection spans 0/1
fill the pass-B region (their Y rows complete after the A region).
PV runs one ki-step behind scores so ScalarE exp is never gated by
the tensor queue. Softmax 1/sum uses reciprocal_approx_fast on the
DMA-broadcast row (the exact DVE reciprocal on [64,512] costs 3.3us
per call and froze the pipeline in v2). PSUM: 2x mg[128,1024] +
2x pv[65,512] + 2x proj[128,512] = exactly 8 banks.
"""

import sys

for _p in ("/opt/trn_rl_repo", "/root/.axon_site"):
    if _p not in sys.path:
        sys.path.insert(0, _p)

import numpy as np
import ml_dtypes

import concourse.bass as bass
import concourse.mybir as mybir
import concourse.tile as tile
from concourse import bacc
from concourse.bass_utils import run_bass_kernel_spmd

B, T, C, H = 2, 2048, 1024, 16
DH = C // H            # 64 head dim
HG = 4                 # heads per core
GW = HG * DH           # 256 features per head group
NB = T // 128          # 16 key chunks
NS = T // 512          # 4 spans
KC = C // 128          # 8 contraction chunks over C
SCALE = 1.0 / float(np.sqrt(DH))
MASKVAL = -1.0e5       # exp((s+MASKVAL)*SCALE) == 0 for any realistic s
N_CORES = 8

F32 = mybir.dt.float32
F16 = mybir.dt.float16
BF16 = mybir.dt.bfloat16
EXP = mybir.ActivationFunctionType.Exp
COPY = mybir.ActivationFunctionType.Copy


def build_nc():
    nc = bacc.Bacc("TRN2", target_bir_lowering=False, debug=False,
                   num_devices=N_CORES)
    xT = nc.declare_dram_parameter("xT", [C, T], BF16, isOutput=False)
    wqT = nc.declare_dram_parameter("wqT", [C, GW], BF16, isOutput=False)
    wkT = nc.declare_dram_parameter("wkT", [C, GW], BF16, isOutput=False)
    wvT = nc.declare_dram_parameter("wvT", [C, GW], BF16, isOutput=False)
    woT = nc.declare_dram_parameter("woT", [GW, C], BF16, isOutput=False)
    outT = nc.declare_dram_parameter("outT", [C, T], F16, isOutput=True)
    s_dram = nc.dram_tensor("s_scratch", [HG, NS, 512], F32)

    with tile.TileContext(nc) as tc:
        with tc.tile_pool(name="pers", bufs=1) as pers, \
             tc.tile_pool(name="PJ", bufs=2, space="PSUM") as PJ, \
             tc.tile_pool(name="MG", bufs=2, space="PSUM") as MG, \
             tc.tile_pool(name="PV", bufs=1, space="PSUM") as PVP, \
             tc.tile_pool(name="PT", bufs=3) as PT, \
             tc.tile_pool(name="NR", bufs=3) as NR, \
             tc.tile_pool(name="OT", bufs=9) as OT:
            # ---- persistent SBUF; DMAs in consumption order.
            # Weights land in one wide tile each (one DMA trigger each);
            # x streams in span-quarters so the first QK group starts
            # after ~1.5MB instead of the full 4MB.
            def load_w(dram, nch, ncol, tag):
                t = pers.tile([128, nch * ncol], BF16, tag=tag, name=tag)
                nc.gpsimd.dma_start(
                    out=t.rearrange("p (k g) -> p k g", g=ncol),
                    in_=dram.rearrange("(k p) g -> p k g", p=128))
                return [t[:, i * ncol:(i + 1) * ncol] for i in range(nch)]

            wk_t = load_w(wkT, KC, GW, "wkall")
            wq_t = load_w(wqT, KC, GW, "wqall")
            xts = [pers.tile([128, T], BF16, tag=f"xT{i}", name=f"xT{i}")
                   for i in range(KC)]
            for s in range(NS):
                cols = slice(s * 512, (s + 1) * 512)
                for i in range(KC):
                    nc.sync.dma_start(out=xts[i][:, cols],
                                      in_=xT[i * 128:(i + 1) * 128, cols])
            wv_t = load_w(wvT, KC, GW, "wvall")
            wo_t = load_w(woT, 2, C, "woall")

            qts = [pers.tile([128, T], BF16, tag=f"qT{m}", name=f"qT{m}")
                   for m in range(2)]
            kts = [pers.tile([128, T], BF16, tag=f"kT{m}", name=f"kT{m}")
                   for m in range(2)]
            yts = [pers.tile([128, T], BF16, tag=f"yT{m}", name=f"yT{m}")
                   for m in range(2)]
            vts = [pers.tile([128, HG * 65], BF16, tag=f"V{tb}", name=f"V{tb}")
                   for tb in range(NB)]

            # identity (bf16) and causal-mask pre-write tile:
            # maskM[i,j] = MASKVAL where j<i (query j < key i) else 0
            ident = pers.tile([128, 128], BF16, tag="ident", name="ident")
            nc.gpsimd.memset(ident, 1.0)
            nc.gpsimd.affine_select(
                out=ident, in_=ident, compare_op=mybir.AluOpType.is_ge,
                fill=0.0, base=0, pattern=[[1, 128]], channel_multiplier=-1)
            nc.gpsimd.affine_select(
                out=ident, in_=ident, compare_op=mybir.AluOpType.is_ge,
                fill=0.0, base=0, pattern=[[-1, 128]], channel_multiplier=1)
            maskM = pers.tile([128, 128], BF16, tag="maskM", name="maskM")
            nc.gpsimd.memset(maskM, MASKVAL)
            nc.gpsimd.affine_select(
                out=maskM, in_=maskM, compare_op=mybir.AluOpType.is_ge,
                fill=0.0, base=-1, pattern=[[-1, 128]], channel_multiplier=1)
            ones4 = pers.tile([128, 4], BF16, tag="ones4", name="ones4")
            for j in range(4):
                nc.scalar.activation(
                    out=ones4[:, j:j + 1],
                    in_=nc.const_aps.tensor(1.0, [128, 1]), func=COPY)
            # ones columns of the V tiles are static: write them once
            for tb in range(NB):
                nc.vector.tensor_copy(
                    out=vts[tb].rearrange("p (h c) -> p h c", c=65)[:, :, 64],
                    in_=ones4)

            # ---- emission helpers ----
            def qk_group(wt, dst, m, s):
                """One projection accumulation group: dst[:, s*512:...]"""
                ps = PJ.tile([128, 512], F32, tag="pj", name="pj")
                for k in range(KC):
                    nc.tensor.matmul(
                        ps, wt[k][:, m * 128:(m + 1) * 128],
                        xts[k][:, s * 512:(s + 1) * 512],
                        start=(k == 0), stop=(k == KC - 1))
                nc.vector.tensor_copy(
                    out=dst[:, s * 512:(s + 1) * 512], in_=ps)

            def v_group(tb):
                """V for key chunk tb in natural [t, d] layout (strided cast)."""
                vps = PJ.tile([128, 512], F32, tag="pj", name="pj")
                for k in range(KC):
                    nc.tensor.matmul(
                        vps[:, 0:GW], xts[k][:, tb * 128:(tb + 1) * 128],
                        wv_t[k], start=(k == 0), stop=(k == KC - 1))
                nc.vector.tensor_copy(
                    out=vts[tb].rearrange("p (h c) -> p h c", c=65)[:, :, 0:64],
                    in_=vps.rearrange("p (h c) -> p h c", c=64)[:, 0:4, :])

            def op_group(m, gs):
                """Output projection for block m, span gs -> OT staging."""
                op = PJ.tile([128, 512], F32, tag="pj", name="pj")
                for j in range(2):
                    nc.tensor.matmul(
                        op, wo_t[j][:, m * 128:(m + 1) * 128],
                        yts[j][:, gs * 512:(gs + 1) * 512],
                        start=(j == 0), stop=(j == 1))
                half = gs // 2
                ot = ot_tiles[m][half]
                if ot is None:
                    ot = OT.tile([128, 1024], F16, tag="ot", name="ot")
                    ot_tiles[m][half] = ot
                nc.vector.tensor_copy(
                    out=ot[:, (gs % 2) * 512:(gs % 2 + 1) * 512], in_=op)
                if gs % 2 == 1:
                    nc.sync.dma_start(
                        out=outT[m * 128:(m + 1) * 128,
                                 half * 1024:(half + 1) * 1024],
                        in_=ot)
                    ot_tiles[m][half] = None

            ot_tiles = [[None, None] for _ in range(8)]

            def norm_span(h, gs, pvt):
                """Normalize completed span: yts <- pv[0:64] / rowsum."""
                m, po = h // 2, (h % 2) * 64
                yv = NR.tile([65, 512], F32, tag="yv", name="yv")
                nc.vector.tensor_copy(out=yv, in_=pvt)
                nc.gpsimd.dma_start(out=s_dram[h, gs, :], in_=yv[64:65, :])
                sb = NR.tile([64, 512], F32, tag="sb", name="sb")
                ssl = s_dram[h, gs, :]
                nc.gpsimd.dma_start(
                    out=sb,
                    in_=bass.AP(tensor=ssl.tensor, offset=ssl.offset,
                                ap=[[0, 64]] + list(ssl.ap)))
                rb = NR.tile([64, 512], F32, tag="rb", name="rb")
                nc.vector.reciprocal_approx_fast(out=rb, in_=sb)
                nc.vector.tensor_mul(
                    out=yts[m][po:po + 64, gs * 512:(gs + 1) * 512],
                    in0=yv[0:64, :], in1=rb)

            # ---- attention for one head, one query-span-pair pass.
            # PV trails scores by one ki step so exp never gates the
            # tensor queue (filler + PV(n-1) + S(n+1) run under exp(n)).
            # ki can run descending so the pass ENDS on its widest
            # strokes, keeping the PE dense across pass boundaries
            # (has_written accumulate-where-set / overwrite-where-clear
            # makes narrow-first PV accumulation correct).
            def attn_pass(h, qpass, fillers, descending=True):
                m, po = h // 2, (h % 2) * 64
                qt, kt = qts[m], kts[m]
                qbase = qpass * 1024
                ki_hi = 8 if qpass == 0 else 16
                ki_order = (list(range(ki_hi - 1, -1, -1)) if descending
                            else list(range(ki_hi)))
                pva = PVP.tile([65, 512], F32, tag="pva", name="pva")
                pvb = PVP.tile([65, 512], F32, tag="pvb", name="pvb")
                pv = (pva, pvb)
                pend = None  # (ki, pt) awaiting PV emission

                def emit_pv(ki, pt):
                    for sp in range(2):
                        gs = qpass * 2 + sp
                        last_ki = 4 * gs + 3
                        if ki > last_ki:
                            continue
                        lo = sp * 512
                        l = max(lo, max(0, 128 * ki - qbase))
                        if l >= lo + 512:
                            continue
                        if descending:
                            first = min(last_ki, ki_hi - 1)
                            st, fin = (ki == first), (ki == 0)
                        else:
                            st, fin = (ki == 0), (ki == last_ki)
                        if st and l > lo:
                            # first (narrowest) write must cover the whole
                            # span uniformly: zero the invalid pt columns
                            # and go full width (PSUM has_written regions
                            # must be uniform per instruction)
                            nc.gpsimd.memset(pt[:, lo:l], 0.0)
                            l = lo
                        nc.tensor.matmul(
                            pv[sp][:, l - lo:512],
                            vts[ki][:, h * 65:(h + 1) * 65],
                            pt[:, l:lo + 512],
                            start=st, stop=fin)
                        if fin:
                            norm_span(h, gs, pv[sp])

                for ki in ki_order:
                    kcol = 128 * ki
                    w0 = max(0, kcol - qbase)
                    diag = kcol >= qbase
                    mg = MG.tile([128, 1024], F32, tag="mg", name="mg")
                    if diag:
                        nc.tensor.matmul(mg[:, w0:w0 + 128], ident, maskM,
                                         start=True, stop=False)
                        nc.tensor.matmul(
                            mg[:, w0:w0 + 128],
                            kt[po:po + 64, kcol:kcol + 128],
                            qt[po:po + 64, qbase + w0:qbase + w0 + 128],
                            start=False, stop=True)
                        segs = []
                        a = w0 + 128
                        if a < 512:
                            segs.append((a, 512))
                        if max(a, 512) < 1024:
                            segs.append((max(a, 512), 1024))
                    else:
                        segs = [(0, 512), (512, 1024)]
                    for (lo, hi) in segs:
                        nc.tensor.matmul(
                            mg[:, lo:hi],
                            kt[po:po + 64, kcol:kcol + 128],
                            qt[po:po + 64, qbase + lo:qbase + hi],
                            start=True, stop=True)
                    pt = PT.tile([128, 1024], BF16, tag="pt", name="pt")
                    nc.scalar.activation(out=pt[:, w0:1024], in_=mg[:, w0:1024],
                                         func=EXP, scale=SCALE)
                    if fillers:
                        fillers.pop(0)()
                    if pend is not None:
                        emit_pv(*pend)
                    pend = (ki, pt)
                if pend is not None:
                    emit_pv(*pend)

            # ---- schedule ----
            # QK projections for head pair 0
            for s in range(NS):
                qk_group(wk_t, kts[0], 0, s)
            for s in range(NS):
                qk_group(wq_t, qts[0], 0, s)
            v_group(0)
            v_group(1)

            # pass-A region fillers. Dependencies: h0-A consumes V(ki) at
            # step ki+1 so V(2..7) must ride h0-A itself; QK for head
            # pair 1 must complete before h2-A's first scores matmul;
            # V(8..15) is only consumed in the pass-B region.
            fa_lists = [
                [(lambda tb: (lambda: v_group(tb)))(tb) for tb in range(2, 8)],
                [(lambda s: (lambda: qk_group(wk_t, kts[1], 1, s)))(s)
                 for s in range(NS)] +
                [(lambda s: (lambda: qk_group(wq_t, qts[1], 1, s)))(s)
                 for s in range(NS)],
                [(lambda tb: (lambda: v_group(tb)))(tb) for tb in range(8, 12)],
                [(lambda tb: (lambda: v_group(tb)))(tb) for tb in range(12, 16)],
            ]

            # h0-A ascending: its own V fillers feed its PV steps
            attn_pass(0, 0, fa_lists[0], descending=False)
            for h in range(1, HG):
                attn_pass(h, 0, fa_lists[h])

            # pass-B region fillers: out-proj spans 0,1 (one per 4 steps)
            fb = [(lambda m, gs: (lambda: op_group(m, gs)))(m, gs)
                  for m in range(8) for gs in range(2)]
            fb_lists = []
            fi = 0
            for h in range(HG):
                sub = []
                for step in range(16):
                    if step % 4 == 1 and fi < len(fb):
                        sub.append(fb[fi])
                        fi += 1
                fb_lists.append(sub)

            # interleave: a pass consumes its filler list one per ki step
            for h in range(HG):
                attn_pass(h, 1, fb_lists[h])

            # tail: all span-2 groups first (their norms are done well
            # before the last head's span-3 chain), then span 3 — the
            # per-m interleave head-of-line blocked on the final norm.
            for m in range(8):
                op_group(m, 2)
            for m in range(8):
                op_group(m, 3)
    nc.compile()
    return nc


_NC_CACHE = None


def _get_nc():
    global _NC_CACHE
    if _NC_CACHE is None:
        _NC_CACHE = build_nc()
    return _NC_CACHE


def make_in_maps(x, wq, wk, wv, wo):
    BF = ml_dtypes.bfloat16
    x = np.asarray(x, dtype=np.float32)
    wq = np.asarray(wq, dtype=np.float32)
    wk = np.asarray(wk, dtype=np.float32)
    wv = np.asarray(wv, dtype=np.float32)
    wo = np.asarray(wo, dtype=np.float32)
    in_maps = []
    for core in range(N_CORES):
        b, g = core // HG, core % HG
        rows = slice(g * GW, (g + 1) * GW)
        in_maps.append({
            "xT": np.ascontiguousarray(x[b].T).astype(BF),
            "wqT": np.ascontiguousarray(wq[rows, :].T).astype(BF),
            "wkT": np.ascontiguousarray(wk[rows, :].T).astype(BF),
            "wvT": np.ascontiguousarray(wv[rows, :].T).astype(BF),
            "woT": np.ascontiguousarray(wo[:, rows].T).astype(BF),
        })
    return in_maps


def run(x, wq, wk, wv, wo, trace=False, tmpdir=None):
    nc = _get_nc()
    in_maps = make_in_maps(x, wq, wk, wv, wo)
    res = run_bass_kernel_spmd(nc, in_maps, core_ids=list(range(N_CORES)),
                               trace=trace, tmpdir=tmpdir)
    out = np.zeros((B, T, C), dtype=np.float32)
    for core in range(N_CORES):
        out[core // HG] += res.results[core]["outT"].T.astype(np.float32)
    return out, res


def kernel(x, wq, wk, wv, wo):
    out, _ = run(x, wq, wk, wv, wo)
    return out


# revision 18
# speedup vs baseline: 1.1678x; 1.0322x over previous
"""Causal self-attention on 8 TRN2 NeuronCores — v3 (pipelined, bf16).

Problem: x[2,2048,1024], wq/wk/wv/wo[1024,1024] (nn.Linear convention,
out = y @ W.T), H=16 heads, D=64, causal softmax, f32.

Sharding: tensor-parallel over heads x data-parallel over batch.
Core i handles batch b=i//4 and head group g=i%4 (4 heads each);
each core returns an f16 partial output projection and the host sums
the 4 partials per batch in f32.

Design: everything bf16 on device; causal mask folded into PSUM by an
identity-matmul pre-write of -1e5 (start=True) that the scores matmul
accumulates onto (start=False), so exp feeds PV directly; attention
runs one head at a time in two query-span-pair passes, ordered
h0A..h3A then h0B..h3B so independent matmul work exists everywhere:
V/QK projections fill the pass-A region, output-projection spans 0/1
fill the pass-B region (their Y rows complete after the A region).
PV runs one ki-step behind scores so ScalarE exp is never gated by
the tensor queue. Softmax 1/sum uses reciprocal_approx_fast on the
DMA-broadcast row (the exact DVE reciprocal on [64,512] costs 3.3us
per call and froze the pipeline in v2). PSUM: 2x mg[128,1024] +
2x pv[65,512] + 2x proj[128,512] = exactly 8 banks.
"""

import sys

for _p in ("/opt/trn_rl_repo", "/root/.axon_site"):
    if _p not in sys.path:
        sys.path.insert(0, _p)

import numpy as np
import ml_dtypes

import concourse.bass as bass
import concourse.mybir as mybir
import concourse.tile as tile
from concourse import bacc
from concourse.bass_utils import run_bass_kernel_spmd

B, T, C, H = 2, 2048, 1024, 16
DH = C // H            # 64 head dim
HG = 4                 # heads per core
GW = HG * DH           # 256 features per head group
NB = T // 128          # 16 key chunks
NS = T // 512          # 4 spans
KC = C // 128          # 8 contraction chunks over C
SCALE = 1.0 / float(np.sqrt(DH))
MASKVAL = -1.0e5       # exp((s+MASKVAL)*SCALE) == 0 for any realistic s
N_CORES = 8

F32 = mybir.dt.float32
F16 = mybir.dt.float16
BF16 = mybir.dt.bfloat16
EXP = mybir.ActivationFunctionType.Exp
COPY = mybir.ActivationFunctionType.Copy


def build_nc():
    nc = bacc.Bacc("TRN2", target_bir_lowering=False, debug=False,
                   num_devices=N_CORES)
    xT = nc.declare_dram_parameter("xT", [C, T], BF16, isOutput=False)
    wqT = nc.declare_dram_parameter("wqT", [C, GW], BF16, isOutput=False)
    wkT = nc.declare_dram_parameter("wkT", [C, GW], BF16, isOutput=False)
    wvT = nc.declare_dram_parameter("wvT", [C, GW], BF16, isOutput=False)
    woT = nc.declare_dram_parameter("woT", [GW, C], BF16, isOutput=False)
    outT = nc.declare_dram_parameter("outT", [C, T], F16, isOutput=True)
    s_dram = nc.dram_tensor("s_scratch", [HG, NS, 512], F32)

    with tile.TileContext(nc) as tc:
        with tc.tile_pool(name="pers", bufs=1) as pers, \
             tc.tile_pool(name="PJ", bufs=2, space="PSUM") as PJ, \
             tc.tile_pool(name="MG", bufs=2, space="PSUM") as MG, \
             tc.tile_pool(name="PV", bufs=1, space="PSUM") as PVP, \
             tc.tile_pool(name="PT", bufs=3) as PT, \
             tc.tile_pool(name="NR", bufs=3) as NR, \
             tc.tile_pool(name="OT", bufs=9) as OT:
            # ---- persistent SBUF; DMAs in consumption order.
            # Weights land in one wide tile each (one DMA trigger each);
            # x streams in span-quarters so the first QK group starts
            # after ~1.5MB instead of the full 4MB.
            def load_w(dram, nch, ncol, tag):
                t = pers.tile([128, nch * ncol], BF16, tag=tag, name=tag)
                nc.gpsimd.dma_start(
                    out=t.rearrange("p (k g) -> p k g", g=ncol),
                    in_=dram.rearrange("(k p) g -> p k g", p=128))
                return [t[:, i * ncol:(i + 1) * ncol] for i in range(nch)]

            wk_t = load_w(wkT, KC, GW, "wkall")
            wq_t = load_w(wqT, KC, GW, "wqall")
            xts = [pers.tile([128, T], BF16, tag=f"xT{i}", name=f"xT{i}")
                   for i in range(KC)]
            for s in range(NS):
                cols = slice(s * 512, (s + 1) * 512)
                for i in range(KC):
                    eng = nc.sync if i % 2 == 0 else nc.scalar
                    eng.dma_start(out=xts[i][:, cols],
                                  in_=xT[i * 128:(i + 1) * 128, cols])
            wv_t = load_w(wvT, KC, GW, "wvall")
            wo_t = load_w(woT, 2, C, "woall")

            qts = [pers.tile([128, T], BF16, tag=f"qT{m}", name=f"qT{m}")
                   for m in range(2)]
            kts = [pers.tile([128, T], BF16, tag=f"kT{m}", name=f"kT{m}")
                   for m in range(2)]
            yts = [pers.tile([128, T], BF16, tag=f"yT{m}", name=f"yT{m}")
                   for m in range(2)]
            vts = [pers.tile([128, HG * 65], BF16, tag=f"V{tb}", name=f"V{tb}")
                   for tb in range(NB)]

            # identity (bf16) and causal-mask pre-write tile:
            # maskM[i,j] = MASKVAL where j<i (query j < key i) else 0
            ident = pers.tile([128, 128], BF16, tag="ident", name="ident")
            nc.gpsimd.memset(ident, 1.0)
            nc.gpsimd.affine_select(
                out=ident, in_=ident, compare_op=mybir.AluOpType.is_ge,
                fill=0.0, base=0, pattern=[[1, 128]], channel_multiplier=-1)
            nc.gpsimd.affine_select(
                out=ident, in_=ident, compare_op=mybir.AluOpType.is_ge,
                fill=0.0, base=0, pattern=[[-1, 128]], channel_multiplier=1)
            maskM = pers.tile([128, 128], BF16, tag="maskM", name="maskM")
            nc.gpsimd.memset(maskM, MASKVAL)
            nc.gpsimd.affine_select(
                out=maskM, in_=maskM, compare_op=mybir.AluOpType.is_ge,
                fill=0.0, base=-1, pattern=[[-1, 128]], channel_multiplier=1)
            ones4 = pers.tile([128, 4], BF16, tag="ones4", name="ones4")
            for j in range(4):
                nc.scalar.activation(
                    out=ones4[:, j:j + 1],
                    in_=nc.const_aps.tensor(1.0, [128, 1]), func=COPY)
            # ones columns of the V tiles are static: write them once
            for tb in range(NB):
                nc.vector.tensor_copy(
                    out=vts[tb].rearrange("p (h c) -> p h c", c=65)[:, :, 64],
                    in_=ones4)

            # ---- emission helpers ----
            def qk_group(wt, dst, m, s):
                """One projection accumulation group: dst[:, s*512:...]"""
                ps = PJ.tile([128, 512], F32, tag="pj", name="pj")
                for k in range(KC):
                    nc.tensor.matmul(
                        ps, wt[k][:, m * 128:(m + 1) * 128],
                        xts[k][:, s * 512:(s + 1) * 512],
                        start=(k == 0), stop=(k == KC - 1))
                nc.vector.tensor_copy(
                    out=dst[:, s * 512:(s + 1) * 512], in_=ps)

            def v_group(tb):
                """V for key chunk tb in natural [t, d] layout (strided cast)."""
                vps = PJ.tile([128, 512], F32, tag="pj", name="pj")
                for k in range(KC):
                    nc.tensor.matmul(
                        vps[:, 0:GW], xts[k][:, tb * 128:(tb + 1) * 128],
                        wv_t[k], start=(k == 0), stop=(k == KC - 1))
                nc.vector.tensor_copy(
                    out=vts[tb].rearrange("p (h c) -> p h c", c=65)[:, :, 0:64],
                    in_=vps.rearrange("p (h c) -> p h c", c=64)[:, 0:4, :])

            def op_group(m, gs):
                """Output projection for block m, span gs -> OT staging."""
                op = PJ.tile([128, 512], F32, tag="pj", name="pj")
                for j in range(2):
                    nc.tensor.matmul(
                        op, wo_t[j][:, m * 128:(m + 1) * 128],
                        yts[j][:, gs * 512:(gs + 1) * 512],
                        start=(j == 0), stop=(j == 1))
                half = gs // 2
                ot = ot_tiles[m][half]
                if ot is None:
                    ot = OT.tile([128, 1024], F16, tag="ot", name="ot")
                    ot_tiles[m][half] = ot
                nc.vector.tensor_copy(
                    out=ot[:, (gs % 2) * 512:(gs % 2 + 1) * 512], in_=op)
                if gs % 2 == 1:
                    nc.sync.dma_start(
                        out=outT[m * 128:(m + 1) * 128,
                                 half * 1024:(half + 1) * 1024],
                        in_=ot)
                    ot_tiles[m][half] = None

            ot_tiles = [[None, None] for _ in range(8)]

            def norm_span(h, gs, pvt):
                """Normalize completed span: yts <- pv[0:64] / rowsum."""
                m, po = h // 2, (h % 2) * 64
                yv = NR.tile([65, 512], F32, tag="yv", name="yv")
                nc.vector.tensor_copy(out=yv, in_=pvt)
                nc.gpsimd.dma_start(out=s_dram[h, gs, :], in_=yv[64:65, :])
                sb = NR.tile([64, 512], F32, tag="sb", name="sb")
                ssl = s_dram[h, gs, :]
                nc.gpsimd.dma_start(
                    out=sb,
                    in_=bass.AP(tensor=ssl.tensor, offset=ssl.offset,
                                ap=[[0, 64]] + list(ssl.ap)))
                rb = NR.tile([64, 512], F32, tag="rb", name="rb")
                nc.vector.reciprocal_approx_fast(out=rb, in_=sb)
                nc.vector.tensor_mul(
                    out=yts[m][po:po + 64, gs * 512:(gs + 1) * 512],
                    in0=yv[0:64, :], in1=rb)

            # ---- attention for one head, one query-span-pair pass.
            # PV trails scores by one ki step so exp never gates the
            # tensor queue (filler + PV(n-1) + S(n+1) run under exp(n)).
            # ki can run descending so the pass ENDS on its widest
            # strokes, keeping the PE dense across pass boundaries
            # (has_written accumulate-where-set / overwrite-where-clear
            # makes narrow-first PV accumulation correct).
            def attn_pass(h, qpass, fillers, descending=True):
                m, po = h // 2, (h % 2) * 64
                qt, kt = qts[m], kts[m]
                qbase = qpass * 1024
                ki_hi = 8 if qpass == 0 else 16
                ki_order = (list(range(ki_hi - 1, -1, -1)) if descending
                            else list(range(ki_hi)))
                pva = PVP.tile([65, 512], F32, tag="pva", name="pva")
                pvb = PVP.tile([65, 512], F32, tag="pvb", name="pvb")
                pv = (pva, pvb)
                pend = None  # (ki, pt) awaiting PV emission

                def emit_pv(ki, pt):
                    for sp in range(2):
                        gs = qpass * 2 + sp
                        last_ki = 4 * gs + 3
                        if ki > last_ki:
                            continue
                        lo = sp * 512
                        l = max(lo, max(0, 128 * ki - qbase))
                        if l >= lo + 512:
                            continue
                        if descending:
                            first = min(last_ki, ki_hi - 1)
                            st, fin = (ki == first), (ki == 0)
                        else:
                            st, fin = (ki == 0), (ki == last_ki)
                        if st and l > lo:
                            # first (narrowest) write must cover the whole
                            # span uniformly: zero the invalid pt columns
                            # and go full width (PSUM has_written regions
                            # must be uniform per instruction)
                            nc.gpsimd.memset(pt[:, lo:l], 0.0)
                            l = lo
                        nc.tensor.matmul(
                            pv[sp][:, l - lo:512],
                            vts[ki][:, h * 65:(h + 1) * 65],
                            pt[:, l:lo + 512],
                            start=st, stop=fin)
                        if fin:
                            norm_span(h, gs, pv[sp])

                for ki in ki_order:
                    kcol = 128 * ki
                    w0 = max(0, kcol - qbase)
                    diag = kcol >= qbase
                    mg = MG.tile([128, 1024], F32, tag="mg", name="mg")
                    if diag:
                        nc.tensor.matmul(mg[:, w0:w0 + 128], ident, maskM,
                                         start=True, stop=False)
                        nc.tensor.matmul(
                            mg[:, w0:w0 + 128],
                            kt[po:po + 64, kcol:kcol + 128],
                            qt[po:po + 64, qbase + w0:qbase + w0 + 128],
                            start=False, stop=True)
                        segs = []
                        a = w0 + 128
                        if a < 512:
                            segs.append((a, 512))
                        if max(a, 512) < 1024:
                            segs.append((max(a, 512), 1024))
                    else:
                        segs = [(0, 512), (512, 1024)]
                    for (lo, hi) in segs:
                        nc.tensor.matmul(
                            mg[:, lo:hi],
                            kt[po:po + 64, kcol:kcol + 128],
                            qt[po:po + 64, qbase + lo:qbase + hi],
                            start=True, stop=True)
                    pt = PT.tile([128, 1024], BF16, tag="pt", name="pt")
                    nc.scalar.activation(out=pt[:, w0:1024], in_=mg[:, w0:1024],
                                         func=EXP, scale=SCALE)
                    if fillers:
                        fillers.pop(0)()
                    if pend is not None:
                        emit_pv(*pend)
                    pend = (ki, pt)
                if pend is not None:
                    emit_pv(*pend)

            # ---- schedule ----
            # QK projections for head pair 0
            for s in range(NS):
                qk_group(wk_t, kts[0], 0, s)
            for s in range(NS):
                qk_group(wq_t, qts[0], 0, s)
            v_group(0)
            v_group(1)

            # pass-A region fillers. Dependencies: h0-A consumes V(ki) at
            # step ki+1 so V(2..7) must ride h0-A itself; QK for head
            # pair 1 must complete before h2-A's first scores matmul;
            # V(8..15) is only consumed in the pass-B region.
            fa_lists = [
                [(lambda tb: (lambda: v_group(tb)))(tb) for tb in range(2, 8)],
                [(lambda s: (lambda: qk_group(wk_t, kts[1], 1, s)))(s)
                 for s in range(NS)] +
                [(lambda s: (lambda: qk_group(wq_t, qts[1], 1, s)))(s)
                 for s in range(NS)],
                [(lambda tb: (lambda: v_group(tb)))(tb) for tb in range(8, 12)],
                [(lambda tb: (lambda: v_group(tb)))(tb) for tb in range(12, 16)],
            ]

            # h0-A ascending: its own V fillers feed its PV steps
            attn_pass(0, 0, fa_lists[0], descending=False)
            for h in range(1, HG):
                attn_pass(h, 0, fa_lists[h])

            # pass-B region fillers: out-proj spans 0,1, split into 1-MM
            # sub-fillers so a filler step stays under the exp period
            def op_sub_fillers(m, gs):
                cell = {}

                def a():
                    op = PJ.tile([128, 512], F32, tag="pj", name="pj")
                    nc.tensor.matmul(
                        op, wo_t[0][:, m * 128:(m + 1) * 128],
                        yts[0][:, gs * 512:(gs + 1) * 512],
                        start=True, stop=False)
                    cell["op"] = op

                def b():
                    op = cell["op"]
                    nc.tensor.matmul(
                        op, wo_t[1][:, m * 128:(m + 1) * 128],
                        yts[1][:, gs * 512:(gs + 1) * 512],
                        start=False, stop=True)
                    half = gs // 2
                    ot = ot_tiles[m][half]
                    if ot is None:
                        ot = OT.tile([128, 1024], F16, tag="ot", name="ot")
                        ot_tiles[m][half] = ot
                    nc.vector.tensor_copy(
                        out=ot[:, (gs % 2) * 512:(gs % 2 + 1) * 512], in_=op)
                    if gs % 2 == 1:
                        nc.sync.dma_start(
                            out=outT[m * 128:(m + 1) * 128,
                                     half * 1024:(half + 1) * 1024],
                            in_=ot)
                        ot_tiles[m][half] = None

                return [a, b]

            fb = []
            for m in range(8):
                for gs in range(2):
                    fb += op_sub_fillers(m, gs)
            fb_lists = [fb[i * 8:(i + 1) * 8] for i in range(HG)]

            # interleave: a pass consumes its filler list one per ki step.
            # h3-B runs ascending so span 2 finalizes at ki=11 — its norm
            # chain completes while ki 12..15 run, and the tail out-proj
            # starts immediately instead of idling the PE behind the last
            # norm (which is what made the v3/v4 tail ~25us).
            attn_pass(0, 1, fb_lists[0])
            attn_pass(1, 1, fb_lists[1])
            attn_pass(2, 1, fb_lists[2])
            attn_pass(3, 1, fb_lists[3], descending=False)

            # tail: all span-2 groups first (their norms are done well
            # before the last head's span-3 chain), then span 3 — the
            # per-m interleave head-of-line blocked on the final norm.
            for m in range(8):
                op_group(m, 2)
            for m in range(8):
                op_group(m, 3)
    nc.compile()
    return nc


_NC_CACHE = None


def _get_nc():
    global _NC_CACHE
    if _NC_CACHE is None:
        _NC_CACHE = build_nc()
    return _NC_CACHE


def make_in_maps(x, wq, wk, wv, wo):
    BF = ml_dtypes.bfloat16
    x = np.asarray(x, dtype=np.float32)
    wq = np.asarray(wq, dtype=np.float32)
    wk = np.asarray(wk, dtype=np.float32)
    wv = np.asarray(wv, dtype=np.float32)
    wo = np.asarray(wo, dtype=np.float32)
    in_maps = []
    for core in range(N_CORES):
        b, g = core // HG, core % HG
        rows = slice(g * GW, (g + 1) * GW)
        in_maps.append({
            "xT": np.ascontiguousarray(x[b].T).astype(BF),
            "wqT": np.ascontiguousarray(wq[rows, :].T).astype(BF),
            "wkT": np.ascontiguousarray(wk[rows, :].T).astype(BF),
            "wvT": np.ascontiguousarray(wv[rows, :].T).astype(BF),
            "woT": np.ascontiguousarray(wo[:, rows].T).astype(BF),
        })
    return in_maps


def run(x, wq, wk, wv, wo, trace=False, tmpdir=None):
    nc = _get_nc()
    in_maps = make_in_maps(x, wq, wk, wv, wo)
    res = run_bass_kernel_spmd(nc, in_maps, core_ids=list(range(N_CORES)),
                               trace=trace, tmpdir=tmpdir)
    out = np.zeros((B, T, C), dtype=np.float32)
    for core in range(N_CORES):
        out[core // HG] += res.results[core]["outT"].T.astype(np.float32)
    return out, res


def kernel(x, wq, wk, wv, wo):
    out, _ = run(x, wq, wk, wv, wo)
    return out


# revision 20
# speedup vs baseline: 1.2336x; 1.0563x over previous
"""Causal self-attention on 8 TRN2 NeuronCores — v3 (pipelined, bf16).

Problem: x[2,2048,1024], wq/wk/wv/wo[1024,1024] (nn.Linear convention,
out = y @ W.T), H=16 heads, D=64, causal softmax, f32.

Sharding: tensor-parallel over heads x data-parallel over batch.
Core i handles batch b=i//4 and head group g=i%4 (4 heads each);
each core returns an f16 partial output projection and the host sums
the 4 partials per batch in f32.

Design: everything bf16 on device; causal mask folded into PSUM by an
identity-matmul pre-write of -1e5 (start=True) that the scores matmul
accumulates onto (start=False), so exp feeds PV directly; attention
runs one head at a time in two query-span-pair passes, ordered
h0A..h3A then h0B..h3B so independent matmul work exists everywhere:
V/QK projections fill the pass-A region, output-projection spans 0/1
fill the pass-B region (their Y rows complete after the A region).
PV runs one ki-step behind scores so ScalarE exp is never gated by
the tensor queue. Softmax 1/sum uses reciprocal_approx_fast on the
DMA-broadcast row (the exact DVE reciprocal on [64,512] costs 3.3us
per call and froze the pipeline in v2). PSUM: 2x mg[128,1024] +
2x pv[65,512] + 2x proj[128,512] = exactly 8 banks.
"""

import sys

for _p in ("/opt/trn_rl_repo", "/root/.axon_site"):
    if _p not in sys.path:
        sys.path.insert(0, _p)

import numpy as np
import ml_dtypes

import concourse.bass as bass
import concourse.mybir as mybir
import concourse.tile as tile
from concourse import bacc
from concourse.bass_utils import run_bass_kernel_spmd

B, T, C, H = 2, 2048, 1024, 16
DH = C // H            # 64 head dim
HG = 4                 # heads per core
GW = HG * DH           # 256 features per head group
NB = T // 128          # 16 key chunks
NS = T // 512          # 4 spans
KC = C // 128          # 8 contraction chunks over C
SCALE = 1.0 / float(np.sqrt(DH))
MASKVAL = -1.0e5       # exp((s+MASKVAL)*SCALE) == 0 for any realistic s
N_CORES = 8

F32 = mybir.dt.float32
F16 = mybir.dt.float16
BF16 = mybir.dt.bfloat16
EXP = mybir.ActivationFunctionType.Exp
COPY = mybir.ActivationFunctionType.Copy


def build_nc():
    nc = bacc.Bacc("TRN2", target_bir_lowering=False, debug=False,
                   num_devices=N_CORES)
    xT = nc.declare_dram_parameter("xT", [C, T], BF16, isOutput=False)
    wqT = nc.declare_dram_parameter("wqT", [C, GW], BF16, isOutput=False)
    wkT = nc.declare_dram_parameter("wkT", [C, GW], BF16, isOutput=False)
    wvT = nc.declare_dram_parameter("wvT", [C, GW], BF16, isOutput=False)
    woT = nc.declare_dram_parameter("woT", [GW, C], BF16, isOutput=False)
    outT = nc.declare_dram_parameter("outT", [C, T], F16, isOutput=True)
    s_dram = nc.dram_tensor("s_scratch", [HG, NS, 512], F32)

    with tile.TileContext(nc) as tc:
        with tc.tile_pool(name="pers", bufs=1) as pers, \
             tc.tile_pool(name="PJ", bufs=2, space="PSUM") as PJ, \
             tc.tile_pool(name="MG", bufs=2, space="PSUM") as MG, \
             tc.tile_pool(name="PV", bufs=1, space="PSUM") as PVP, \
             tc.tile_pool(name="PT", bufs=3) as PT, \
             tc.tile_pool(name="NR", bufs=3) as NR, \
             tc.tile_pool(name="OT", bufs=9) as OT:
            # ---- persistent SBUF; DMAs in consumption order.
            # Weights land in one wide tile each (one DMA trigger each);
            # x streams in span-quarters so the first QK group starts
            # after ~1.5MB instead of the full 4MB.
            def load_w(dram, nch, ncol, tag):
                t = pers.tile([128, nch * ncol], BF16, tag=tag, name=tag)
                nc.gpsimd.dma_start(
                    out=t.rearrange("p (k g) -> p k g", g=ncol),
                    in_=dram.rearrange("(k p) g -> p k g", p=128))
                return [t[:, i * ncol:(i + 1) * ncol] for i in range(nch)]

            wk_t = load_w(wkT, KC, GW, "wkall")
            wq_t = load_w(wqT, KC, GW, "wqall")
            xts = [pers.tile([128, T], BF16, tag=f"xT{i}", name=f"xT{i}")
                   for i in range(KC)]
            for s in range(NS):
                cols = slice(s * 512, (s + 1) * 512)
                for i in range(KC):
                    eng = nc.sync if i % 2 == 0 else nc.scalar
                    eng.dma_start(out=xts[i][:, cols],
                                  in_=xT[i * 128:(i + 1) * 128, cols])
            wv_t = load_w(wvT, KC, GW, "wvall")
            wo_t = load_w(woT, 2, C, "woall")

            qts = [pers.tile([128, T], BF16, tag=f"qT{m}", name=f"qT{m}")
                   for m in range(2)]
            kts = [pers.tile([128, T], BF16, tag=f"kT{m}", name=f"kT{m}")
                   for m in range(2)]
            yts = [pers.tile([128, T], BF16, tag=f"yT{m}", name=f"yT{m}")
                   for m in range(2)]
            vts = [pers.tile([128, HG * 65], BF16, tag=f"V{tb}", name=f"V{tb}")
                   for tb in range(NB)]

            # identity (bf16) and causal-mask pre-write tile:
            # maskM[i,j] = MASKVAL where j<i (query j < key i) else 0
            ident = pers.tile([128, 128], BF16, tag="ident", name="ident")
            nc.gpsimd.memset(ident, 1.0)
            nc.gpsimd.affine_select(
                out=ident, in_=ident, compare_op=mybir.AluOpType.is_ge,
                fill=0.0, base=0, pattern=[[1, 128]], channel_multiplier=-1)
            nc.gpsimd.affine_select(
                out=ident, in_=ident, compare_op=mybir.AluOpType.is_ge,
                fill=0.0, base=0, pattern=[[-1, 128]], channel_multiplier=1)
            maskM = pers.tile([128, 128], BF16, tag="maskM", name="maskM")
            nc.gpsimd.memset(maskM, MASKVAL)
            nc.gpsimd.affine_select(
                out=maskM, in_=maskM, compare_op=mybir.AluOpType.is_ge,
                fill=0.0, base=-1, pattern=[[-1, 128]], channel_multiplier=1)
            ones4 = pers.tile([128, 4], BF16, tag="ones4", name="ones4")
            for j in range(4):
                nc.scalar.activation(
                    out=ones4[:, j:j + 1],
                    in_=nc.const_aps.tensor(1.0, [128, 1]), func=COPY)
            # ones columns of the V tiles are static: write them once
            for tb in range(NB):
                nc.vector.tensor_copy(
                    out=vts[tb].rearrange("p (h c) -> p h c", c=65)[:, :, 64],
                    in_=ones4)

            # ---- emission helpers ----
            def qk_group(wt, dst, m, s):
                """One projection accumulation group: dst[:, s*512:...]"""
                ps = PJ.tile([128, 512], F32, tag="pj", name="pj")
                for k in range(KC):
                    nc.tensor.matmul(
                        ps, wt[k][:, m * 128:(m + 1) * 128],
                        xts[k][:, s * 512:(s + 1) * 512],
                        start=(k == 0), stop=(k == KC - 1))
                nc.vector.tensor_copy(
                    out=dst[:, s * 512:(s + 1) * 512], in_=ps)

            def v_group(tb):
                """V for key chunk tb in natural [t, d] layout (strided cast)."""
                vps = PJ.tile([128, 512], F32, tag="pj", name="pj")
                for k in range(KC):
                    nc.tensor.matmul(
                        vps[:, 0:GW], xts[k][:, tb * 128:(tb + 1) * 128],
                        wv_t[k], start=(k == 0), stop=(k == KC - 1))
                nc.vector.tensor_copy(
                    out=vts[tb].rearrange("p (h c) -> p h c", c=65)[:, :, 0:64],
                    in_=vps.rearrange("p (h c) -> p h c", c=64)[:, 0:4, :])

            def op_group(m, gs, use_scalar=False):
                """Output projection for block m, span gs -> OT staging.

                use_scalar routes the PSUM->SBUF cast through ScalarE —
                it is idle after the last exp, while DVE is busy with the
                norm chains and would stall the PJ ring."""
                op = PJ.tile([128, 512], F32, tag="pj", name="pj")
                for j in range(2):
                    nc.tensor.matmul(
                        op, wo_t[j][:, m * 128:(m + 1) * 128],
                        yts[j][:, gs * 512:(gs + 1) * 512],
                        start=(j == 0), stop=(j == 1))
                half = gs // 2
                ot = ot_tiles[m][half]
                if ot is None:
                    ot = OT.tile([128, 1024], F16, tag="ot", name="ot")
                    ot_tiles[m][half] = ot
                if use_scalar:
                    nc.scalar.activation(
                        out=ot[:, (gs % 2) * 512:(gs % 2 + 1) * 512], in_=op,
                        func=COPY)
                else:
                    nc.vector.tensor_copy(
                        out=ot[:, (gs % 2) * 512:(gs % 2 + 1) * 512], in_=op)
                if gs % 2 == 1:
                    nc.sync.dma_start(
                        out=outT[m * 128:(m + 1) * 128,
                                 half * 1024:(half + 1) * 1024],
                        in_=ot)
                    ot_tiles[m][half] = None

            ot_tiles = [[None, None] for _ in range(8)]

            def norm_span(h, gs, pvt):
                """Normalize completed span: yts <- pv[0:64] / rowsum."""
                m, po = h // 2, (h % 2) * 64
                yv = NR.tile([65, 512], F32, tag="yv", name="yv")
                nc.vector.tensor_copy(out=yv, in_=pvt)
                nc.gpsimd.dma_start(out=s_dram[h, gs, :], in_=yv[64:65, :])
                sb = NR.tile([64, 512], F32, tag="sb", name="sb")
                ssl = s_dram[h, gs, :]
                nc.gpsimd.dma_start(
                    out=sb,
                    in_=bass.AP(tensor=ssl.tensor, offset=ssl.offset,
                                ap=[[0, 64]] + list(ssl.ap)))
                rb = NR.tile([64, 512], F32, tag="rb", name="rb")
                nc.vector.reciprocal_approx_fast(out=rb, in_=sb)
                nc.vector.tensor_mul(
                    out=yts[m][po:po + 64, gs * 512:(gs + 1) * 512],
                    in0=yv[0:64, :], in1=rb)

            # ---- attention for one head, one query-span-pair pass.
            # PV trails scores by one ki step so exp never gates the
            # tensor queue (filler + PV(n-1) + S(n+1) run under exp(n)).
            # ki can run descending so the pass ENDS on its widest
            # strokes, keeping the PE dense across pass boundaries
            # (has_written accumulate-where-set / overwrite-where-clear
            # makes narrow-first PV accumulation correct).
            def attn_pass(h, qpass, fillers, descending=True):
                m, po = h // 2, (h % 2) * 64
                qt, kt = qts[m], kts[m]
                qbase = qpass * 1024
                ki_hi = 8 if qpass == 0 else 16
                ki_order = (list(range(ki_hi - 1, -1, -1)) if descending
                            else list(range(ki_hi)))
                pva = PVP.tile([65, 512], F32, tag="pva", name="pva")
                pvb = PVP.tile([65, 512], F32, tag="pvb", name="pvb")
                pv = (pva, pvb)
                pend = None  # (ki, pt) awaiting PV emission

                def emit_pv(ki, pt):
                    for sp in range(2):
                        gs = qpass * 2 + sp
                        last_ki = 4 * gs + 3
                        if ki > last_ki:
                            continue
                        lo = sp * 512
                        l = max(lo, max(0, 128 * ki - qbase))
                        if l >= lo + 512:
                            continue
                        if descending:
                            first = min(last_ki, ki_hi - 1)
                            st, fin = (ki == first), (ki == 0)
                        else:
                            st, fin = (ki == 0), (ki == last_ki)
                        if st and l > lo:
                            # first (narrowest) write must cover the whole
                            # span uniformly: zero the invalid pt columns
                            # and go full width (PSUM has_written regions
                            # must be uniform per instruction)
                            nc.gpsimd.memset(pt[:, lo:l], 0.0)
                            l = lo
                        nc.tensor.matmul(
                            pv[sp][:, l - lo:512],
                            vts[ki][:, h * 65:(h + 1) * 65],
                            pt[:, l:lo + 512],
                            start=st, stop=fin)
                        if fin:
                            norm_span(h, gs, pv[sp])

                for ki in ki_order:
                    kcol = 128 * ki
                    w0 = max(0, kcol - qbase)
                    diag = kcol >= qbase
                    mg = MG.tile([128, 1024], F32, tag="mg", name="mg")
                    if diag:
                        nc.tensor.matmul(mg[:, w0:w0 + 128], ident, maskM,
                                         start=True, stop=False)
                        nc.tensor.matmul(
                            mg[:, w0:w0 + 128],
                            kt[po:po + 64, kcol:kcol + 128],
                            qt[po:po + 64, qbase + w0:qbase + w0 + 128],
                            start=False, stop=True)
                        segs = []
                        a = w0 + 128
                        if a < 512:
                            segs.append((a, 512))
                        if max(a, 512) < 1024:
                            segs.append((max(a, 512), 1024))
                    else:
                        segs = [(0, 512), (512, 1024)]
                    for (lo, hi) in segs:
                        nc.tensor.matmul(
                            mg[:, lo:hi],
                            kt[po:po + 64, kcol:kcol + 128],
                            qt[po:po + 64, qbase + lo:qbase + hi],
                            start=True, stop=True)
                    pt = PT.tile([128, 1024], BF16, tag="pt", name="pt")
                    nc.scalar.activation(out=pt[:, w0:1024], in_=mg[:, w0:1024],
                                         func=EXP, scale=SCALE)
                    if fillers:
                        fillers.pop(0)()
                    if pend is not None:
                        emit_pv(*pend)
                    pend = (ki, pt)
                if pend is not None:
                    emit_pv(*pend)

            # ---- schedule ----
            # QK projections for head pair 0
            for s in range(NS):
                qk_group(wk_t, kts[0], 0, s)
            for s in range(NS):
                qk_group(wq_t, qts[0], 0, s)
            v_group(0)
            v_group(1)

            # pass-A region fillers. Dependencies: h0-A consumes V(ki) at
            # step ki+1 so V(2..7) must ride h0-A itself; QK for head
            # pair 1 must complete before h2-A's first scores matmul;
            # V(8..15) is only consumed in the pass-B region.
            fa_lists = [
                [(lambda tb: (lambda: v_group(tb)))(tb) for tb in range(2, 8)],
                [(lambda s: (lambda: qk_group(wk_t, kts[1], 1, s)))(s)
                 for s in range(NS)] +
                [(lambda s: (lambda: qk_group(wq_t, qts[1], 1, s)))(s)
                 for s in range(NS)],
                [(lambda tb: (lambda: v_group(tb)))(tb) for tb in range(8, 12)],
                [(lambda tb: (lambda: v_group(tb)))(tb) for tb in range(12, 16)],
            ]

            # h0-A ascending: its own V fillers feed its PV steps
            attn_pass(0, 0, fa_lists[0], descending=False)
            for h in range(1, HG):
                attn_pass(h, 0, fa_lists[h])

            # pass-B region fillers: out-proj spans 0,1, split into 1-MM
            # sub-fillers so a filler step stays under the exp period
            def op_sub_fillers(m, gs):
                cell = {}

                def a():
                    op = PJ.tile([128, 512], F32, tag="pj", name="pj")
                    nc.tensor.matmul(
                        op, wo_t[0][:, m * 128:(m + 1) * 128],
                        yts[0][:, gs * 512:(gs + 1) * 512],
                        start=True, stop=False)
                    cell["op"] = op

                def b():
                    op = cell["op"]
                    nc.tensor.matmul(
                        op, wo_t[1][:, m * 128:(m + 1) * 128],
                        yts[1][:, gs * 512:(gs + 1) * 512],
                        start=False, stop=True)
                    half = gs // 2
                    ot = ot_tiles[m][half]
                    if ot is None:
                        ot = OT.tile([128, 1024], F16, tag="ot", name="ot")
                        ot_tiles[m][half] = ot
                    nc.vector.tensor_copy(
                        out=ot[:, (gs % 2) * 512:(gs % 2 + 1) * 512], in_=op)
                    if gs % 2 == 1:
                        nc.sync.dma_start(
                            out=outT[m * 128:(m + 1) * 128,
                                     half * 1024:(half + 1) * 1024],
                            in_=ot)
                        ot_tiles[m][half] = None

                return [a, b]

            fb = []
            for m in range(8):
                for gs in range(2):
                    fb += op_sub_fillers(m, gs)
            fb_lists = [fb[i * 8:(i + 1) * 8] for i in range(HG)]

            # interleave: a pass consumes its filler list one per ki step.
            # h3-B runs ascending so span 2 finalizes at ki=11 — its norm
            # chain completes while ki 12..15 run, and the tail out-proj
            # starts immediately instead of idling the PE behind the last
            # norm (which is what made the v3/v4 tail ~25us).
            attn_pass(0, 1, fb_lists[0])
            attn_pass(1, 1, fb_lists[1])
            attn_pass(2, 1, fb_lists[2])
            attn_pass(3, 1, fb_lists[3], descending=False)

            # tail: all span-2 groups first (their norms are done well
            # before the last head's span-3 chain), then span 3 — the
            # per-m interleave head-of-line blocked on the final norm.
            for m in range(8):
                op_group(m, 2, use_scalar=True)
            for m in range(8):
                op_group(m, 3, use_scalar=True)
    nc.compile()
    return nc


_NC_CACHE = None


def _get_nc():
    global _NC_CACHE
    if _NC_CACHE is None:
        _NC_CACHE = build_nc()
    return _NC_CACHE


def make_in_maps(x, wq, wk, wv, wo):
    BF = ml_dtypes.bfloat16
    x = np.asarray(x, dtype=np.float32)
    wq = np.asarray(wq, dtype=np.float32)
    wk = np.asarray(wk, dtype=np.float32)
    wv = np.asarray(wv, dtype=np.float32)
    wo = np.asarray(wo, dtype=np.float32)
    in_maps = []
    for core in range(N_CORES):
        b, g = core // HG, core % HG
        rows = slice(g * GW, (g + 1) * GW)
        in_maps.append({
            "xT": np.ascontiguousarray(x[b].T).astype(BF),
            "wqT": np.ascontiguousarray(wq[rows, :].T).astype(BF),
            "wkT": np.ascontiguousarray(wk[rows, :].T).astype(BF),
            "wvT": np.ascontiguousarray(wv[rows, :].T).astype(BF),
            "woT": np.ascontiguousarray(wo[:, rows].T).astype(BF),
        })
    return in_maps


def run(x, wq, wk, wv, wo, trace=False, tmpdir=None):
    nc = _get_nc()
    in_maps = make_in_maps(x, wq, wk, wv, wo)
    res = run_bass_kernel_spmd(nc, in_maps, core_ids=list(range(N_CORES)),
                               trace=trace, tmpdir=tmpdir)
    out = np.zeros((B, T, C), dtype=np.float32)
    for core in range(N_CORES):
        out[core // HG] += res.results[core]["outT"].T.astype(np.float32)
    return out, res


def kernel(x, wq, wk, wv, wo):
    out, _ = run(x, wq, wk, wv, wo)
    return out
